# revision 18
# baseline (speedup 1.0000x reference)
"""KANLinear forward on 8 Trainium2 NeuronCores (data-parallel over batch).

Factorization
-------------
reference computes, per token row x (after clip/renorm preprocessing):
    y = silu(x) @ base_weight.T + einsum('big,oig->bo', bsplines(x), sw*scaler)

The cubic B-spline bases over the uniform grid (h=0.4, knots -2.2..2.2) are
    B_g(x) = N3(s - g),  s = 2.5*x + 5.5,  g = 0..7
with N3 the cardinal cubic B-spline on [0,4].  Both the spline einsum and the
silu base path collapse into a single K=4096 bf16 matmul per 128-row output
tile:  K rows hold (sw[o,i,g]*scaler[o,i])/6 + c_g*base_weight (silu is
projected onto the spline basis; c = lstsq fit under the clipped-N(0,1) input
measure).  The features 6*N3(s-g) are produced two ways in parallel:
  * g < N_DVE: two fused custom-DVE instructions (8-stage pipelines, PageIdx
    paging over g) via 6*N3(t) = relu(min(t,4-t))^3 - 4*relu(min(t,4-t)-1)^3
    (the 4x is folded as z = 2*zp^3, w = wp^3 - z - z; no gamma pre-scale)
  * g >= N_DVE: one ScalarE ACTIVATE per g through a custom ACT spline table
    (the stock `sin` entry of silu_and_others is rewritten so that
    activation(Sin, scale=0.125, bias=(9.5-g)/8) returns 6*N3(s-g) exactly)
Batch dim (16384) is sharded 2048 rows/core; weights are replicated.

Scheduling (v3): steady-state chunk-matmuls run r-major (all 4 o-tiles of a
K-chunk before the next chunk) so the DVE-produced chunks (consumed last in
CHUNK_ORDER) get ~4us more slack -- this removes the periodic 2-slot PE
stalls v2 had.  The last (bs,it) step flushes each acc[o] to SBUF/DRAM as
soon as its own accumulation stops, hiding the drain under the remaining
o-tiles' matmuls.  The first x half-tile DMA is the sync queue's first op
(128 cols so its completion semaphore posts ASAP), V's first two K-chunks
ride the otherwise-idle scalar hardware queue, and the PE HAM warm-up burst
is sized so the queue frees right as the first real features land.
"""

import hashlib
import os
import shutil
import tempfile

import numpy as np

B, IN_F, OUT_F, NG = 16384, 512, 512, 8
N_CORES = 8
BPC = B // N_CORES            # batch rows per core
BS = 512                      # batch-column slice processed per step
N_BS = BPC // BS              # 4 slices
N_IT = IN_F // 128            # 4 input-feature partition tiles
KC = N_IT * NG                # 32 K-chunks of 128
N_DVE = 3                     # bases 0..N_DVE-1 on VectorE; rest on ScalarE ACT
CHUNK_ORDER = list(range(N_DVE, NG)) + list(range(N_DVE))  # ACT chunks first
N_WARM = 28                   # PE HAM-warmup throwaway matmuls

_state = {}


# --------------------------------------------------------------------------
# Custom ACT table: hijack `sin` in silu_and_others to evaluate 6*N3(8u-4).
# Verified-on-HW stock mapping: ctrl entry = 42+(exp-116); entry 52 (binade
# [0.5,1)) has 8 sub-buckets of width 1/16 at buckets 1034..1041; bucket
# eval is y = d0+(u-x0)(d1+(u-x0)(d2+(u-x0)d3)); |u|<2^-11 -> bucket
# 1075/1076 (sign-folded); large |u| -> 1077/1078.  Buckets 1020..1078 are
# sin-private; everything else (silu, copy, ...) is untouched.
# --------------------------------------------------------------------------
def _n3_6_coeffs(j):
    return {
        0: [0.0, 0.0, 0.0, 1.0],
        1: [1.0, 3.0, 3.0, -3.0],
        2: [4.0, 0.0, -6.0, 3.0],
        3: [1.0, -3.0, 3.0, -1.0],
    }[j]


def _compose(c, scale, shift):
    c0, c1, c2, c3 = c
    return [
        c0 + c1 * shift + c2 * shift**2 + c3 * shift**3,
        scale * (c1 + 2 * c2 * shift + 3 * c3 * shift**2),
        scale**2 * (c2 + 3 * c3 * shift),
        scale**3 * c3,
    ]


def _build_custom_act_root():
    if "act_root" in _state:
        return _state["act_root"], _state["act_sig"]
    from neuronxcc.driver.Job import Job
    from neuronxcc.driver.jobs.support.FindActInfo import findActInfoFile

    src_json = findActInfoFile(Job.getPackageDir(), "gen3")
    src_dir = os.path.dirname(src_json)
    dst_dir = tempfile.mkdtemp(prefix="kan_act_root_")
    for f in os.listdir(src_dir):
        shutil.copy(os.path.join(src_dir, f), os.path.join(dst_dir, f))
    for f in os.listdir(dst_dir):
        os.chmod(os.path.join(dst_dir, f), 0o644)

    bkt_path = os.path.join(dst_dir, "silu_and_others_bkt.bin")
    bkt = np.fromfile(bkt_path, dtype=np.float32).reshape(-1, 8).copy()
    bkt[1020:1079] = 0.0
    for k in range(8):
        x0 = 0.5 + k / 16.0 + 1.0 / 32.0
        j = k // 2
        q = _compose(_n3_6_coeffs(j), 8.0, 8.0 * x0 - 4.0 - j)
        bkt[1034 + k] = [q[0], q[1], q[2], q[3], x0, 0.0, 0.0, 0.0]
    bkt.tofile(bkt_path)

    sig = hashlib.sha256(open(bkt_path, "rb").read()).hexdigest()[:10]
    path = os.path.join(dst_dir, "act_info.json")
    os.environ["BASS_ACT_ROOT_JSON_PATH"] = path
    _state["act_root"] = path
    _state["act_sig"] = sig
    return path, sig


# --------------------------------------------------------------------------
# Custom DVE ops
# --------------------------------------------------------------------------
def _register_ops():
    if "ops" in _state:
        return _state["ops"]
    import concourse.dve_ops as dve_ops
    from concourse.dve_spec import (
        Spec, Src0, Src1, C0, C1, C2, One, PageIdx, relu, sq, maxx, minn, lower,
    )
    from concourse.dve_uop import DveOpSpec

    def page_idx_np(in0, s0, s1):
        S = in0.shape[1]
        return (s0 + s1 * np.arange(S, dtype=np.float64)).astype(np.float32)[
            None, :, None
        ]

    def pre_ref(in0, in1, s0, s1, imm2):
        t = np.minimum(np.maximum(in0, np.float32(s0)), np.float32(s1))
        t = ((t + np.float32(1)) - np.float32(1)).astype(np.float32)
        return (t * np.float32(imm2)).astype(np.float32)

    def z_ref(in0, in1, s0, s1, imm2):
        t = (in0 + page_idx_np(in0, s0, s1)).astype(np.float32)
        m = np.minimum(t, np.float32(imm2) - t)
        zp = np.maximum(m, np.float32(0))
        d = (zp + zp).astype(np.float32)
        return ((d * d) * zp).astype(np.float32)

    def w_ref(in0, in1, s0, s1, imm2):
        t = (in0 + page_idx_np(in0, s0, s1)).astype(np.float32)
        m = np.minimum(t, np.float32(imm2) - t)
        wp = np.maximum(m, np.float32(0))
        ww = (wp * wp).astype(np.float32)
        return ((ww * wp) - in1).astype(np.float32)

    pre_spec = Spec(
        body=((minn(maxx(Src0, C0), C1) + One) - One) * C2, reference=pre_ref
    )
    # zp = relu(min(t-1, 3-t)) = relu(min(tz, 2-tz)), tz = xs + 4.5 - g;
    # z = 4*zp^3 via sq(zp+zp)*zp; w = relu(min(t,4-t))^3 - z.  No gamma
    # pre-scale needed -- both ops read xs directly.
    _pgz = PageIdx(C0, C1)
    _tz = Src0 + _pgz
    _zp = relu(minn(_tz, C2 - _tz))
    z_spec = Spec(body=sq(_zp + _zp) * _zp, reference=z_ref)
    _pgw = PageIdx(C0, C1)
    _tw = Src0 + _pgw
    _wp = relu(minn(_tw, C2 - _tw))
    w_spec = Spec(body=sq(_wp) * _wp - Src1, reference=w_ref)

    ops = {}
    for name, spec, subdim in (
        ("KAN_PRE", pre_spec, False),
        ("KAN_Z3", z_spec, True),
        ("KAN_W3", w_spec, True),
    ):
        if name in dve_ops._SUB_OPCODE_FOR_NAME:
            ops[name] = next(o for o in dve_ops.OPS if o.name == name)
            continue
        row = dve_ops._CUSTOM_DVE_ROW_BASE + len(dve_ops.OPS)
        assert row < 0x20, "custom-DVE row overflow"
        shas = {}
        for ver in ("v3", "v4"):
            try:
                tmp = DveOpSpec(
                    name=name, opcode=row, uops=lower(spec, ver=ver),
                    rd1_en=dve_ops.has_src1(spec),
                )
                shas[ver] = tmp.sha(ver)
            except Exception:
                pass
        op = dve_ops.DveOp(name, spec, subdim=subdim, uops_sha=shas)
        dve_ops.OPS.append(op)
        dve_ops._SUB_OPCODE_FOR_NAME[name] = row
        dve_ops.CUSTOM_DVE_SPECS[name] = spec
        ops[name] = op
    _state["ops"] = ops
    return ops


# --------------------------------------------------------------------------
# Kernel build
# --------------------------------------------------------------------------
def _build_kernel():
    if "nc" in _state:
        return _state["nc"]
    import concourse.bacc as bacc
    import concourse.mybir as mybir
    import concourse.tile as tile
    from concourse.bass import ts

    _build_custom_act_root()
    ops = _register_ops()
    f32 = mybir.dt.float32
    bf16 = mybir.dt.bfloat16
    AF = mybir.ActivationFunctionType

    nc = bacc.Bacc()

    # x is shipped bf16 (halves input DMA traffic; adds ~5e-4 rel err) and
    # V partition-major so every DMA slice is contiguous per partition.
    xT = nc.dram_tensor("xT", [IN_F, BPC], bf16, kind="ExternalInput")
    V = nc.dram_tensor("V", [128, KC * OUT_F], bf16, kind="ExternalInput")
    yT = nc.dram_tensor("yT", [OUT_F, BPC], bf16, kind="ExternalOutput")

    # V DMA slices (in units of K-chunks, consumption order), all on
    # gpsimd's software-dynamic queue: fine-grained early slices so chunk r
    # lands before the first-step matmul that consumes it.  Slice 0 is the
    # queue's first op (it absorbs the queue spin-up itself).
    # Slice pacing: gpsimd's software queue posts completions ~2-3us apart,
    # too slow for the chunks the first-step matmuls consume at a ~1.1us
    # cadence -- so only chunk 0 (needed first, gpsimd's first sem lands
    # ~10.1us) and the late bulk ride gpsimd; chunks 1-7 ride the sync
    # hardware queue (one completion per ~0.7us) interleaved with the x
    # tiles.
    V_SLICES_GP = [(0, 1), (6, 8), (8, 16), (16, 24), (24, 32)]
    V_SLICES_SYNC_PRE = [(1, 2), (2, 4), (4, 6)]

    def flush_one(nc, ysb_pool, acc, o, bs, split):
        ysb = ysb_pool.tile([128, BS], bf16, name=f"ysb{o}")
        if split:
            for hc0, hc1 in ((0, BS // 2), (BS // 2, BS)):
                nc.scalar.copy(ysb[:, hc0:hc1], acc[:, hc0:hc1])
                nc.sync.dma_start(
                    yT[ts(o, 128), bs * BS + hc0 : bs * BS + hc1],
                    ysb[:, hc0:hc1],
                )
        else:
            nc.scalar.copy(ysb[:], acc[:])
            nc.sync.dma_start(yT[ts(o, 128), ts(bs, BS)], ysb[:])

    with tile.TileContext(nc) as tc:
        with (
            tc.tile_pool(name="vpool", bufs=1) as vpool,
            tc.tile_pool(name="const", bufs=1) as const_pool,
            tc.tile_pool(name="warm", bufs=1) as warm_pool,
            tc.tile_pool(name="xin", bufs=3) as xin_pool,
            tc.tile_pool(name="xs", bufs=3) as xs_pool,
            tc.tile_pool(name="z3", bufs=2) as z3_pool,
            tc.tile_pool(name="feat", bufs=8) as feat_pool,
            tc.tile_pool(name="ysb", bufs=4) as ysb_pool,
            tc.tile_pool(name="psum", bufs=8, space="PSUM") as psum_pool,
        ):
            v_sb = vpool.tile([128, KC, OUT_F], bf16)
            v_view = V[:].rearrange("p (kc o) -> p kc o", kc=KC)

            # The first x half-tile is the sync hardware queue's FIRST op:
            # its completion semaphore gates the whole feature->matmul chain.
            FIRST_HALVES = [(0, BS // 2), (BS // 2, BS)]
            xin0 = xin_pool.tile([128, BS], bf16, name="xin0")
            for c0, c1 in FIRST_HALVES:
                nc.sync.dma_start(xin0[:, c0:c1], xT[0:128, c0:c1])

            # warm-matmul scratch memset on gpsimd (its earliest user slot),
            # then the V stream.
            warm = warm_pool.tile([128, 129], bf16, name="warmw")
            nc.gpsimd.memset(warm[:, 0:128], 0.0)
            for a, b in V_SLICES_GP:
                nc.gpsimd.dma_start(v_sb[:, a:b, :], v_view[:, a:b, :])
            for a, b in V_SLICES_SYNC_PRE:
                nc.sync.dma_start(v_sb[:, a:b, :], v_view[:, a:b, :])

            # Kick the ACT table load for silu_and_others immediately so it
            # overlaps the input DMAs instead of the first feature chain.
            cb = const_pool.tile([128, NG + 1], f32, name="cbias")
            nc.vector.memset(cb[:, NG : NG + 1], 0.0)
            for g in range(N_DVE, NG):
                nc.vector.memset(cb[:, g : g + 1], (9.5 - g) / 8.0)
            nc.vector.memset(warm[:, 128:129], 0.0)
            nc.scalar.activation(
                warm[:, 128:129], warm[:, 128:129], AF.Silu, bias=cb[:, NG : NG + 1]
            )

            # PE HAM warm-up: throwaway matmuls into a scratch PSUM tile so
            # the clock-gate lifts before the real matmul stream begins.
            warm_ps = psum_pool.tile([128, BS], f32, name="warmps", tag="acc")
            for _ in range(N_WARM):
                nc.tensor.matmul(
                    warm_ps[:, 0:128], warm[:, 0:128], warm[:, 0:128],
                    start=True, stop=True,
                )

            pending = None  # (accs, bs) whose y copies are deferred
            for bs in range(N_BS):
                accs = [
                    psum_pool.tile([128, BS], f32, name=f"acc{o}", tag="acc")
                    for o in range(N_IT)
                ]
                for it in range(N_IT):
                    first_step = bs == 0 and it == 0
                    last_step = bs == N_BS - 1 and it == N_IT - 1
                    halves = FIRST_HALVES if first_step else [(0, BS)]
                    if first_step:
                        xin = xin0  # DMA'd in the preamble slot above
                    else:
                        xin = xin_pool.tile([128, BS], bf16)
                        nc.sync.dma_start(xin[:], xT[ts(it, 128), ts(bs, BS)])
                    xs = xs_pool.tile([128, BS], f32)
                    for c0, c1 in halves:
                        nc.vector._custom_dve(
                            ops["KAN_PRE"], out=xs[:, c0:c1], in0=xin[:, c0:c1],
                            s0=-1.1, s1=1.1, imm2=2.5,
                        )
                    ft = feat_pool.tile([128, NG, BS], bf16)
                    # bases N_DVE..7: one ACT spline-table op each (half-0's
                    # five SINs all before half-1's, matching MM consumption)
                    for c0, c1 in halves:
                        for g in range(N_DVE, NG):
                            nc.scalar.activation(
                                ft[:, g, c0:c1], xs[:, c0:c1], AF.Sin,
                                scale=0.125, bias=cb[:, g : g + 1],
                            )
                    # bases 0..N_DVE-1: two fused paged DVE ops (z = 4*zp^3,
                    # w = wp^3 - z), both directly on xs
                    z3 = z3_pool.tile([128, N_DVE, BS], f32)
                    nc.vector._custom_dve(
                        ops["KAN_Z3"],
                        out=z3[:],
                        in0=xs[:].unsqueeze(1).broadcast_to([128, N_DVE, BS]),
                        s0=4.5, s1=-1.0, imm2=2.0,
                    )
                    nc.vector._custom_dve(
                        ops["KAN_W3"],
                        out=ft[:, 0:N_DVE, :],
                        in0=xs[:].unsqueeze(1).broadcast_to([128, N_DVE, BS]),
                        in1=z3[:].rearrange("p s n -> p (s n)"),
                        s0=5.5, s1=-1.0, imm2=4.0,
                    )
                    if last_step:
                        # o-major, column halves; flush each acc[o] as soon
                        # as its own accumulation stops so the PSUM drain
                        # pipelines with the remaining o-tiles' matmuls.
                        # The last o additionally flushes half-0 before
                        # half-1's matmuls run.
                        for o in range(N_IT):
                            last_o = o == N_IT - 1
                            for hc0, hc1 in ((0, BS // 2), (BS // 2, BS)):
                                for r in range(NG):
                                    g = CHUNK_ORDER[r]
                                    nc.tensor.matmul(
                                        accs[o][:, hc0:hc1],
                                        v_sb[:, it * NG + r, ts(o, 128)],
                                        ft[:, g, hc0:hc1],
                                        start=False,
                                        stop=(hc0 > 0 and r == NG - 1),
                                        skip_group_check=True,
                                    )
                            if last_o:
                                # drain via the idle DVE (reacts faster
                                # than ScalarE's queued COPY after the
                                # final matmul); single copy+DMA, no
                                # column split -- a partial read of the
                                # still-accumulating tile serializes the
                                # remaining matmuls (coarse PSUM tracking)
                                ysb3 = ysb_pool.tile(
                                    [128, BS], bf16, name="ysb3"
                                )
                                nc.vector.tensor_copy(ysb3[:], accs[o][:])
                                nc.sync.dma_start(
                                    yT[ts(o, 128), ts(bs, BS)], ysb3[:]
                                )
                            else:
                                flush_one(nc, ysb_pool, accs[o], o, bs,
                                          split=False)
                        continue
                    if first_step:
                        # column-half-major: all ACT ranks of half-0 for all
                        # o first (half-1 features and the DVE chunks are
                        # still being produced), then half-1, then the DVE
                        # ranks full-width.  start=True on each acc's first
                        # MM clears the whole bank, so the later half-1 /
                        # full-width MMs accumulate correctly.
                        for hi, (c0, c1) in enumerate(halves):
                            for r, g in enumerate(CHUNK_ORDER):
                                if g < N_DVE:
                                    continue
                                for o in range(N_IT):
                                    nc.tensor.matmul(
                                        accs[o][:, c0:c1],
                                        v_sb[:, it * NG + r, ts(o, 128)],
                                        ft[:, g, c0:c1],
                                        start=(hi == 0 and r == 0),
                                        stop=False,
                                        skip_group_check=True,
                                    )
                        for r, g in enumerate(CHUNK_ORDER):
                            if g >= N_DVE:
                                continue
                            for o in range(N_IT):
                                nc.tensor.matmul(
                                    accs[o][:],
                                    v_sb[:, it * NG + r, ts(o, 128)],
                                    ft[:, g, :],
                                    start=False, stop=False,
                                    skip_group_check=True,
                                )
                    else:
                        # r-major: all 4 o-tiles of a K-chunk back-to-back,
                        # so the DVE-produced chunks (ranks 5..7) aren't
                        # needed until ~4.3us into the step.
                        for r, g in enumerate(CHUNK_ORDER):
                            for o in range(N_IT):
                                nc.tensor.matmul(
                                    accs[o][:],
                                    v_sb[:, it * NG + r, ts(o, 128)],
                                    ft[:, g, :],
                                    start=(it == 0 and r == 0),
                                    stop=(it == N_IT - 1 and r == NG - 1),
                                    skip_group_check=True,
                                )
                    if it == 0 and pending is not None:
                        paccs, pbs = pending
                        for o in range(N_IT):
                            flush_one(nc, ysb_pool, paccs[o], o, pbs, split=False)
                        pending = None
                pending = (accs, bs)
            # bs == N_BS-1 was flushed inside last_step

    nc.compile()
    _state["nc"] = nc
    return nc


def _silu_in_basis():
    """Project silu(x) on [-1.1, 1.1] onto the 8 B-spline bases, weighted by
    the clipped-N(0,1) input distribution (atoms at the clamp bounds)."""
    from math import erf, sqrt

    def n3(t):
        wp = np.maximum(np.minimum(t, 4 - t), 0.0)
        zp = np.maximum(np.minimum(t - 1, 3 - t), 0.0)
        return (wp**3 - 4 * zp**3) / 6.0

    x = np.linspace(-1.0999, 1.0999, 8001)
    w = np.exp(-x**2 / 2) / np.sqrt(2 * np.pi) * (x[1] - x[0])
    tail = 1 - 0.5 * (1 + erf(1.1 / sqrt(2)))
    X = np.concatenate([x, [-1.1, 1.1]])
    W = np.concatenate([w, [tail, tail]])
    s = 2.5 * X + 5.5
    Bm = np.stack([n3(s - g) for g in range(NG)], axis=-1)
    F = X / (1 + np.exp(-X))
    swr = np.sqrt(W)
    c, *_ = np.linalg.lstsq(Bm * swr[:, None], F * swr, rcond=None)
    return c  # (8,)


def _build_V(base_weight, spline_weight, spline_scaler):
    sw = spline_weight.astype(np.float32) * spline_scaler.astype(np.float32)[:, :, None]
    vs = np.transpose(sw, (2, 1, 0)) / np.float32(6.0)  # [g, i, o]
    bwT = base_weight.astype(np.float32).T  # [i, o]
    c = _silu_in_basis() / 6.0
    V = np.empty((KC * 128, OUT_F), dtype=np.float32)
    for it in range(N_IT):
        isl = slice(it * 128, (it + 1) * 128)
        for r, g in enumerate(CHUNK_ORDER):
            k = it * NG + r
            V[k * 128 : (k + 1) * 128] = vs[g, isl, :] + np.float32(c[g]) * bwT[isl, :]
    # partition-major: [p, kc, o] so each DMA slice is per-partition contiguous
    Vp = V.reshape(KC, 128, OUT_F).transpose(1, 0, 2).reshape(128, KC * OUT_F)
    import ml_dtypes
    return np.ascontiguousarray(Vp.astype(ml_dtypes.bfloat16))


def kernel(x, base_weight, spline_weight, spline_scaler, grid):
    from concourse.bass_utils import run_bass_kernel_spmd

    import ml_dtypes

    nc = _build_kernel()
    Vb = _build_V(base_weight, spline_weight, spline_scaler)
    x = np.asarray(x, dtype=np.float32)
    in_maps = []
    for c in range(N_CORES):
        xTc = np.ascontiguousarray(
            x[c * BPC : (c + 1) * BPC, :].T.astype(ml_dtypes.bfloat16)
        )
        in_maps.append({"xT": xTc, "V": Vb})
    res = run_bass_kernel_spmd(nc, in_maps, core_ids=list(range(N_CORES)))
    y = np.empty((B, OUT_F), dtype=np.float32)
    for c in range(N_CORES):
        y[c * BPC : (c + 1) * BPC, :] = res.results[c]["yT"].T
    return y


# revision 21
# speedup vs baseline: 1.0543x; 1.0543x over previous
"""KANLinear forward on 8 Trainium2 NeuronCores (data-parallel over batch).

Factorization
-------------
reference computes, per token row x (after clip/renorm preprocessing):
    y = silu(x) @ base_weight.T + einsum('big,oig->bo', bsplines(x), sw*scaler)

The cubic B-spline bases over the uniform grid (h=0.4, knots -2.2..2.2) are
    B_g(x) = N3(s - g),  s = 2.5*x + 5.5,  g = 0..7
with N3 the cardinal cubic B-spline on [0,4].  Both the spline einsum and the
silu base path collapse into a single K=4096 bf16 matmul per 128-row output
tile:  K rows hold (sw[o,i,g]*scaler[o,i])/6 + c_g*base_weight (silu is
projected onto the spline basis; c = lstsq fit under the clipped-N(0,1) input
measure).  The features 6*N3(s-g) are produced two ways in parallel:
  * g < N_DVE: two fused custom-DVE instructions (8-stage pipelines, PageIdx
    paging over g) via 6*N3(t) = relu(min(t,4-t))^3 - 4*relu(min(t,4-t)-1)^3
    (the 4x is folded as z = 2*zp^3, w = wp^3 - z - z; no gamma pre-scale)
  * g >= N_DVE: one ScalarE ACTIVATE per g through a custom ACT spline table
    (the stock `sin` entry of silu_and_others is rewritten so that
    activation(Sin, scale=0.125, bias=(9.5-g)/8) returns 6*N3(s-g) exactly)
Batch dim (16384) is sharded 2048 rows/core; weights are replicated.

Scheduling (v3): steady-state chunk-matmuls run r-major (all 4 o-tiles of a
K-chunk before the next chunk) so the DVE-produced chunks (consumed last in
CHUNK_ORDER) get ~4us more slack -- this removes the periodic 2-slot PE
stalls v2 had.  The last (bs,it) step flushes each acc[o] to SBUF/DRAM as
soon as its own accumulation stops, hiding the drain under the remaining
o-tiles' matmuls.  The first x half-tile DMA is the sync queue's first op
(128 cols so its completion semaphore posts ASAP), V's first two K-chunks
ride the otherwise-idle scalar hardware queue, and the PE HAM warm-up burst
is sized so the queue frees right as the first real features land.
"""

import hashlib
import os
import shutil
import tempfile

import numpy as np

B, IN_F, OUT_F, NG = 16384, 512, 512, 8
N_CORES = 8
BPC = B // N_CORES            # batch rows per core
BS = 512                      # batch-column slice processed per step
N_BS = BPC // BS              # 4 slices
N_IT = IN_F // 128            # 4 input-feature partition tiles
KC = N_IT * NG                # 32 K-chunks of 128
N_DVE = 3                     # bases 0..N_DVE-1 on VectorE; rest on ScalarE ACT
CHUNK_ORDER = list(range(N_DVE, NG)) + list(range(N_DVE))  # ACT chunks first
N_WARM = 34                   # PE HAM-warmup throwaway matmuls

_state = {}


# --------------------------------------------------------------------------
# Custom ACT table: hijack `sin` in silu_and_others to evaluate 6*N3(8u-4).
# Verified-on-HW stock mapping: ctrl entry = 42+(exp-116); entry 52 (binade
# [0.5,1)) has 8 sub-buckets of width 1/16 at buckets 1034..1041; bucket
# eval is y = d0+(u-x0)(d1+(u-x0)(d2+(u-x0)d3)); |u|<2^-11 -> bucket
# 1075/1076 (sign-folded); large |u| -> 1077/1078.  Buckets 1020..1078 are
# sin-private; everything else (silu, copy, ...) is untouched.
# --------------------------------------------------------------------------
def _n3_6_coeffs(j):
    return {
        0: [0.0, 0.0, 0.0, 1.0],
        1: [1.0, 3.0, 3.0, -3.0],
        2: [4.0, 0.0, -6.0, 3.0],
        3: [1.0, -3.0, 3.0, -1.0],
    }[j]


def _compose(c, scale, shift):
    c0, c1, c2, c3 = c
    return [
        c0 + c1 * shift + c2 * shift**2 + c3 * shift**3,
        scale * (c1 + 2 * c2 * shift + 3 * c3 * shift**2),
        scale**2 * (c2 + 3 * c3 * shift),
        scale**3 * c3,
    ]


def _build_custom_act_root():
    if "act_root" in _state:
        return _state["act_root"], _state["act_sig"]
    from neuronxcc.driver.Job import Job
    from neuronxcc.driver.jobs.support.FindActInfo import findActInfoFile

    src_json = findActInfoFile(Job.getPackageDir(), "gen3")
    src_dir = os.path.dirname(src_json)
    dst_dir = tempfile.mkdtemp(prefix="kan_act_root_")
    for f in os.listdir(src_dir):
        shutil.copy(os.path.join(src_dir, f), os.path.join(dst_dir, f))
    for f in os.listdir(dst_dir):
        os.chmod(os.path.join(dst_dir, f), 0o644)

    bkt_path = os.path.join(dst_dir, "silu_and_others_bkt.bin")
    bkt = np.fromfile(bkt_path, dtype=np.float32).reshape(-1, 8).copy()
    bkt[1020:1079] = 0.0
    for k in range(8):
        x0 = 0.5 + k / 16.0 + 1.0 / 32.0
        j = k // 2
        q = _compose(_n3_6_coeffs(j), 8.0, 8.0 * x0 - 4.0 - j)
        bkt[1034 + k] = [q[0], q[1], q[2], q[3], x0, 0.0, 0.0, 0.0]
    bkt.tofile(bkt_path)

    sig = hashlib.sha256(open(bkt_path, "rb").read()).hexdigest()[:10]
    path = os.path.join(dst_dir, "act_info.json")
    os.environ["BASS_ACT_ROOT_JSON_PATH"] = path
    _state["act_root"] = path
    _state["act_sig"] = sig
    return path, sig


# --------------------------------------------------------------------------
# Custom DVE ops
# --------------------------------------------------------------------------
def _register_ops():
    if "ops" in _state:
        return _state["ops"]
    import concourse.dve_ops as dve_ops
    from concourse.dve_spec import (
        Spec, Src0, Src1, C0, C1, C2, One, PageIdx, relu, sq, maxx, minn, lower,
    )
    from concourse.dve_uop import DveOpSpec

    def page_idx_np(in0, s0, s1):
        S = in0.shape[1]
        return (s0 + s1 * np.arange(S, dtype=np.float64)).astype(np.float32)[
            None, :, None
        ]

    def pre_ref(in0, in1, s0, s1, imm2):
        t = np.minimum(np.maximum(in0, np.float32(s0)), np.float32(s1))
        t = ((t + np.float32(1)) - np.float32(1)).astype(np.float32)
        return (t * np.float32(imm2)).astype(np.float32)

    def z_ref(in0, in1, s0, s1, imm2):
        t = (in0 + page_idx_np(in0, s0, s1)).astype(np.float32)
        m = np.minimum(t, np.float32(imm2) - t)
        zp = np.maximum(m, np.float32(0))
        d = (zp + zp).astype(np.float32)
        return ((d * d) * zp).astype(np.float32)

    def w_ref(in0, in1, s0, s1, imm2):
        t = (in0 + page_idx_np(in0, s0, s1)).astype(np.float32)
        m = np.minimum(t, np.float32(imm2) - t)
        wp = np.maximum(m, np.float32(0))
        ww = (wp * wp).astype(np.float32)
        return ((ww * wp) - in1).astype(np.float32)

    pre_spec = Spec(
        body=((minn(maxx(Src0, C0), C1) + One) - One) * C2, reference=pre_ref
    )
    # zp = relu(min(t-1, 3-t)) = relu(min(tz, 2-tz)), tz = xs + 4.5 - g;
    # z = 4*zp^3 via sq(zp+zp)*zp; w = relu(min(t,4-t))^3 - z.  No gamma
    # pre-scale needed -- both ops read xs directly.
    _pgz = PageIdx(C0, C1)
    _tz = Src0 + _pgz
    _zp = relu(minn(_tz, C2 - _tz))
    z_spec = Spec(body=sq(_zp + _zp) * _zp, reference=z_ref)
    _pgw = PageIdx(C0, C1)
    _tw = Src0 + _pgw
    _wp = relu(minn(_tw, C2 - _tw))
    w_spec = Spec(body=sq(_wp) * _wp - Src1, reference=w_ref)

    ops = {}
    for name, spec, subdim in (
        ("KAN_PRE", pre_spec, False),
        ("KAN_Z3", z_spec, True),
        ("KAN_W3", w_spec, True),
    ):
        if name in dve_ops._SUB_OPCODE_FOR_NAME:
            ops[name] = next(o for o in dve_ops.OPS if o.name == name)
            continue
        row = dve_ops._CUSTOM_DVE_ROW_BASE + len(dve_ops.OPS)
        assert row < 0x20, "custom-DVE row overflow"
        shas = {}
        for ver in ("v3", "v4"):
            try:
                tmp = DveOpSpec(
                    name=name, opcode=row, uops=lower(spec, ver=ver),
                    rd1_en=dve_ops.has_src1(spec),
                )
                shas[ver] = tmp.sha(ver)
            except Exception:
                pass
        op = dve_ops.DveOp(name, spec, subdim=subdim, uops_sha=shas)
        dve_ops.OPS.append(op)
        dve_ops._SUB_OPCODE_FOR_NAME[name] = row
        dve_ops.CUSTOM_DVE_SPECS[name] = spec
        ops[name] = op
    _state["ops"] = ops
    return ops


# --------------------------------------------------------------------------
# Kernel build
# --------------------------------------------------------------------------
def _build_kernel():
    if "nc" in _state:
        return _state["nc"]
    import concourse.bacc as bacc
    import concourse.mybir as mybir
    import concourse.tile as tile
    from concourse.bass import ts

    _build_custom_act_root()
    ops = _register_ops()
    f32 = mybir.dt.float32
    bf16 = mybir.dt.bfloat16
    AF = mybir.ActivationFunctionType

    nc = bacc.Bacc()

    # x is shipped bf16 (halves input DMA traffic; adds ~5e-4 rel err) and
    # V partition-major so every DMA slice is contiguous per partition.
    xT = nc.dram_tensor("xT", [IN_F, BPC], bf16, kind="ExternalInput")
    V = nc.dram_tensor("V", [128, KC * OUT_F], bf16, kind="ExternalInput")
    yT = nc.dram_tensor("yT", [OUT_F, BPC], bf16, kind="ExternalOutput")

    # V DMA slices (in units of K-chunks, consumption order), all on
    # gpsimd's software-dynamic queue: fine-grained early slices so chunk r
    # lands before the first-step matmul that consumes it.  Slice 0 is the
    # queue's first op (it absorbs the queue spin-up itself).
    # Slice pacing (all on gpsimd -- sharing the sync queue pushes the x
    # tiles late and cascades): gpsimd's first completion lands ~10.1us +
    # wire, the second only ~12.4us, then ~0.7-1us apart.  A 3-chunk first
    # slice covers the first three consumption deadlines (~10.2, 10.9,
    # 11.8us) with one early semaphore.
    V_SLICES_GP = [
        (0, 3), (3, 5), (5, 8), (8, 16), (16, 24), (24, 32),
    ]

    def flush_one(nc, ysb_pool, acc, o, bs, split):
        ysb = ysb_pool.tile([128, BS], bf16, name=f"ysb{o}")
        if split:
            for hc0, hc1 in ((0, BS // 2), (BS // 2, BS)):
                nc.scalar.copy(ysb[:, hc0:hc1], acc[:, hc0:hc1])
                nc.sync.dma_start(
                    yT[ts(o, 128), bs * BS + hc0 : bs * BS + hc1],
                    ysb[:, hc0:hc1],
                )
        else:
            nc.scalar.copy(ysb[:], acc[:])
            nc.sync.dma_start(yT[ts(o, 128), ts(bs, BS)], ysb[:])

    with tile.TileContext(nc) as tc:
        with (
            tc.tile_pool(name="vpool", bufs=1) as vpool,
            tc.tile_pool(name="const", bufs=1) as const_pool,
            tc.tile_pool(name="warm", bufs=1) as warm_pool,
            tc.tile_pool(name="xin", bufs=3) as xin_pool,
            tc.tile_pool(name="xs", bufs=3) as xs_pool,
            tc.tile_pool(name="z3", bufs=2) as z3_pool,
            tc.tile_pool(name="feat", bufs=8) as feat_pool,
            tc.tile_pool(name="ysb", bufs=4) as ysb_pool,
            tc.tile_pool(name="psum", bufs=8, space="PSUM") as psum_pool,
        ):
            v_sb = vpool.tile([128, KC, OUT_F], bf16)
            v_view = V[:].rearrange("p (kc o) -> p kc o", kc=KC)

            # The first x half-tile is the sync hardware queue's FIRST op:
            # its completion semaphore gates the whole feature->matmul chain.
            FIRST_HALVES = [(0, BS // 2), (BS // 2, BS)]
            xin0 = xin_pool.tile([128, BS], bf16, name="xin0")
            for c0, c1 in FIRST_HALVES:
                nc.sync.dma_start(xin0[:, c0:c1], xT[0:128, c0:c1])

            # warm-matmul scratch memset on gpsimd (its earliest user slot),
            # then the V stream.
            warm = warm_pool.tile([128, 129], bf16, name="warmw")
            nc.gpsimd.memset(warm[:, 0:128], 0.0)
            for a, b in V_SLICES_GP:
                nc.gpsimd.dma_start(v_sb[:, a:b, :], v_view[:, a:b, :])

            # Kick the ACT table load for silu_and_others immediately so it
            # overlaps the input DMAs instead of the first feature chain.
            cb = const_pool.tile([128, NG + 1], f32, name="cbias")
            nc.vector.memset(cb[:, NG : NG + 1], 0.0)
            for g in range(N_DVE, NG):
                nc.vector.memset(cb[:, g : g + 1], (9.5 - g) / 8.0)
            nc.vector.memset(warm[:, 128:129], 0.0)
            nc.scalar.activation(
                warm[:, 128:129], warm[:, 128:129], AF.Silu, bias=cb[:, NG : NG + 1]
            )

            # PE HAM warm-up: throwaway matmuls into a scratch PSUM tile so
            # the clock-gate lifts before the real matmul stream begins.
            warm_ps = psum_pool.tile([128, BS], f32, name="warmps", tag="acc")
            for _ in range(N_WARM):
                nc.tensor.matmul(
                    warm_ps[:, 0:128], warm[:, 0:128], warm[:, 0:128],
                    start=True, stop=True,
                )

            pending = None  # (accs, bs) whose y copies are deferred
            for bs in range(N_BS):
                accs = [
                    psum_pool.tile([128, BS], f32, name=f"acc{o}", tag="acc")
                    for o in range(N_IT)
                ]
                for it in range(N_IT):
                    first_step = bs == 0 and it == 0
                    last_step = bs == N_BS - 1 and it == N_IT - 1
                    halves = FIRST_HALVES if first_step else [(0, BS)]
                    if first_step:
                        xin = xin0  # DMA'd in the preamble slot above
                    else:
                        xin = xin_pool.tile([128, BS], bf16)
                        nc.sync.dma_start(xin[:], xT[ts(it, 128), ts(bs, BS)])
                    xs = xs_pool.tile([128, BS], f32)
                    for c0, c1 in halves:
                        nc.vector._custom_dve(
                            ops["KAN_PRE"], out=xs[:, c0:c1], in0=xin[:, c0:c1],
                            s0=-1.1, s1=1.1, imm2=2.5,
                        )
                    ft = feat_pool.tile([128, NG, BS], bf16)
                    # bases N_DVE..7: one ACT spline-table op each (half-0's
                    # five SINs all before half-1's, matching MM consumption)
                    for c0, c1 in halves:
                        for g in range(N_DVE, NG):
                            nc.scalar.activation(
                                ft[:, g, c0:c1], xs[:, c0:c1], AF.Sin,
                                scale=0.125, bias=cb[:, g : g + 1],
                            )
                    # bases 0..N_DVE-1: two fused paged DVE ops (z = 4*zp^3,
                    # w = wp^3 - z), both directly on xs
                    z3 = z3_pool.tile([128, N_DVE, BS], f32)
                    nc.vector._custom_dve(
                        ops["KAN_Z3"],
                        out=z3[:],
                        in0=xs[:].unsqueeze(1).broadcast_to([128, N_DVE, BS]),
                        s0=4.5, s1=-1.0, imm2=2.0,
                    )
                    nc.vector._custom_dve(
                        ops["KAN_W3"],
                        out=ft[:, 0:N_DVE, :],
                        in0=xs[:].unsqueeze(1).broadcast_to([128, N_DVE, BS]),
                        in1=z3[:].rearrange("p s n -> p (s n)"),
                        s0=5.5, s1=-1.0, imm2=4.0,
                    )
                    if last_step:
                        # o-major, column halves; flush each acc[o] as soon
                        # as its own accumulation stops so the PSUM drain
                        # pipelines with the remaining o-tiles' matmuls.
                        # The last o additionally flushes half-0 before
                        # half-1's matmuls run.
                        for o in range(N_IT):
                            last_o = o == N_IT - 1
                            for hc0, hc1 in ((0, BS // 2), (BS // 2, BS)):
                                for r in range(NG):
                                    g = CHUNK_ORDER[r]
                                    nc.tensor.matmul(
                                        accs[o][:, hc0:hc1],
                                        v_sb[:, it * NG + r, ts(o, 128)],
                                        ft[:, g, hc0:hc1],
                                        start=False,
                                        stop=(hc0 > 0 and r == NG - 1),
                                        skip_group_check=True,
                                    )
                            if last_o:
                                # drain via the idle DVE (reacts faster
                                # than ScalarE's queued COPY after the
                                # final matmul); single copy+DMA, no
                                # column split -- a partial read of the
                                # still-accumulating tile serializes the
                                # remaining matmuls (coarse PSUM tracking)
                                ysb3 = ysb_pool.tile(
                                    [128, BS], bf16, name="ysb3"
                                )
                                nc.vector.tensor_copy(ysb3[:], accs[o][:])
                                nc.sync.dma_start(
                                    yT[ts(o, 128), ts(bs, BS)], ysb3[:]
                                )
                            else:
                                flush_one(nc, ysb_pool, accs[o], o, bs,
                                          split=False)
                        continue
                    if first_step:
                        # column-half-major: all ACT ranks of half-0 for all
                        # o first (half-1 features and the DVE chunks are
                        # still being produced), then half-1, then the DVE
                        # ranks full-width.  start=True on each acc's first
                        # MM clears the whole bank, so the later half-1 /
                        # full-width MMs accumulate correctly.
                        for hi, (c0, c1) in enumerate(halves):
                            for r, g in enumerate(CHUNK_ORDER):
                                if g < N_DVE:
                                    continue
                                for o in range(N_IT):
                                    nc.tensor.matmul(
                                        accs[o][:, c0:c1],
                                        v_sb[:, it * NG + r, ts(o, 128)],
                                        ft[:, g, c0:c1],
                                        start=(hi == 0 and r == 0),
                                        stop=False,
                                        skip_group_check=True,
                                    )
                        for r, g in enumerate(CHUNK_ORDER):
                            if g >= N_DVE:
                                continue
                            for o in range(N_IT):
                                nc.tensor.matmul(
                                    accs[o][:],
                                    v_sb[:, it * NG + r, ts(o, 128)],
                                    ft[:, g, :],
                                    start=False, stop=False,
                                    skip_group_check=True,
                                )
                    else:
                        # r-major: all 4 o-tiles of a K-chunk back-to-back,
                        # so the DVE-produced chunks (ranks 5..7) aren't
                        # needed until ~4.3us into the step.
                        for r, g in enumerate(CHUNK_ORDER):
                            for o in range(N_IT):
                                nc.tensor.matmul(
                                    accs[o][:],
                                    v_sb[:, it * NG + r, ts(o, 128)],
                                    ft[:, g, :],
                                    start=(it == 0 and r == 0),
                                    stop=(it == N_IT - 1 and r == NG - 1),
                                    skip_group_check=True,
                                )
                    if it == 0 and pending is not None:
                        paccs, pbs = pending
                        for o in range(N_IT):
                            flush_one(nc, ysb_pool, paccs[o], o, pbs, split=False)
                        pending = None
                pending = (accs, bs)
            # bs == N_BS-1 was flushed inside last_step

    nc.compile()
    _state["nc"] = nc
    return nc


def _silu_in_basis():
    """Project silu(x) on [-1.1, 1.1] onto the 8 B-spline bases, weighted by
    the clipped-N(0,1) input distribution (atoms at the clamp bounds)."""
    from math import erf, sqrt

    def n3(t):
        wp = np.maximum(np.minimum(t, 4 - t), 0.0)
        zp = np.maximum(np.minimum(t - 1, 3 - t), 0.0)
        return (wp**3 - 4 * zp**3) / 6.0

    x = np.linspace(-1.0999, 1.0999, 8001)
    w = np.exp(-x**2 / 2) / np.sqrt(2 * np.pi) * (x[1] - x[0])
    tail = 1 - 0.5 * (1 + erf(1.1 / sqrt(2)))
    X = np.concatenate([x, [-1.1, 1.1]])
    W = np.concatenate([w, [tail, tail]])
    s = 2.5 * X + 5.5
    Bm = np.stack([n3(s - g) for g in range(NG)], axis=-1)
    F = X / (1 + np.exp(-X))
    swr = np.sqrt(W)
    c, *_ = np.linalg.lstsq(Bm * swr[:, None], F * swr, rcond=None)
    return c  # (8,)


def _build_V(base_weight, spline_weight, spline_scaler):
    sw = spline_weight.astype(np.float32) * spline_scaler.astype(np.float32)[:, :, None]
    vs = np.transpose(sw, (2, 1, 0)) / np.float32(6.0)  # [g, i, o]
    bwT = base_weight.astype(np.float32).T  # [i, o]
    c = _silu_in_basis() / 6.0
    V = np.empty((KC * 128, OUT_F), dtype=np.float32)
    for it in range(N_IT):
        isl = slice(it * 128, (it + 1) * 128)
        for r, g in enumerate(CHUNK_ORDER):
            k = it * NG + r
            V[k * 128 : (k + 1) * 128] = vs[g, isl, :] + np.float32(c[g]) * bwT[isl, :]
    # partition-major: [p, kc, o] so each DMA slice is per-partition contiguous
    Vp = V.reshape(KC, 128, OUT_F).transpose(1, 0, 2).reshape(128, KC * OUT_F)
    import ml_dtypes
    return np.ascontiguousarray(Vp.astype(ml_dtypes.bfloat16))


def kernel(x, base_weight, spline_weight, spline_scaler, grid):
    from concourse.bass_utils import run_bass_kernel_spmd

    import ml_dtypes

    nc = _build_kernel()
    Vb = _build_V(base_weight, spline_weight, spline_scaler)
    x = np.asarray(x, dtype=np.float32)
    in_maps = []
    for c in range(N_CORES):
        xTc = np.ascontiguousarray(
            x[c * BPC : (c + 1) * BPC, :].T.astype(ml_dtypes.bfloat16)
        )
        in_maps.append({"xT": xTc, "V": Vb})
    res = run_bass_kernel_spmd(nc, in_maps, core_ids=list(range(N_CORES)))
    y = np.empty((B, OUT_F), dtype=np.float32)
    for c in range(N_CORES):
        y[c * BPC : (c + 1) * BPC, :] = res.results[c]["yT"].T
    return y


# revision 24
# speedup vs baseline: 1.1141x; 1.0567x over previous
"""KANLinear forward on 8 Trainium2 NeuronCores (data-parallel over batch).

Factorization
-------------
reference computes, per token row x (after clip/renorm preprocessing):
    y = silu(x) @ base_weight.T + einsum('big,oig->bo', bsplines(x), sw*scaler)

The cubic B-spline bases over the uniform grid (h=0.4, knots -2.2..2.2) are
    B_g(x) = N3(s - g),  s = 2.5*x + 5.5,  g = 0..7
with N3 the cardinal cubic B-spline on [0,4].  Both the spline einsum and the
silu base path collapse into a single K=4096 bf16 matmul per 128-row output
tile:  K rows hold (sw[o,i,g]*scaler[o,i])/6 + c_g*base_weight (silu is
projected onto the spline basis; c = lstsq fit under the clipped-N(0,1) input
measure).  The features 6*N3(s-g) are produced two ways in parallel:
  * g < N_DVE: two fused custom-DVE instructions (8-stage pipelines, PageIdx
    paging over g) via 6*N3(t) = relu(min(t,4-t))^3 - 4*relu(min(t,4-t)-1)^3
    (the 4x is folded as z = 2*zp^3, w = wp^3 - z - z; no gamma pre-scale)
  * g >= N_DVE: one ScalarE ACTIVATE per g through a custom ACT spline table
    (the stock `sin` entry of silu_and_others is rewritten so that
    activation(Sin, scale=0.125, bias=(9.5-g)/8) returns 6*N3(s-g) exactly)
Batch dim (16384) is sharded 2048 rows/core; weights are replicated.

Scheduling (v3): steady-state chunk-matmuls run r-major (all 4 o-tiles of a
K-chunk before the next chunk) so the DVE-produced chunks (consumed last in
CHUNK_ORDER) get ~4us more slack -- this removes the periodic 2-slot PE
stalls v2 had.  The last (bs,it) step flushes each acc[o] to SBUF/DRAM as
soon as its own accumulation stops, hiding the drain under the remaining
o-tiles' matmuls.  The first x half-tile DMA is the sync queue's first op
(128 cols so its completion semaphore posts ASAP), V's first two K-chunks
ride the otherwise-idle scalar hardware queue, and the PE HAM warm-up burst
is sized so the queue frees right as the first real features land.
"""

import hashlib
import os
import shutil
import tempfile

import numpy as np

B, IN_F, OUT_F, NG = 16384, 512, 512, 8
N_CORES = 8
BPC = B // N_CORES            # batch rows per core
BS = 512                      # batch-column slice processed per step
N_BS = BPC // BS              # 4 slices
N_IT = IN_F // 128            # 4 input-feature partition tiles
KC = N_IT * NG                # 32 K-chunks of 128
N_DVE = 3                     # bases 0..N_DVE-1 on VectorE; rest on ScalarE ACT
CHUNK_ORDER = list(range(N_DVE, NG)) + list(range(N_DVE))  # ACT chunks first
N_WARM = 34                   # PE HAM-warmup throwaway matmuls

_state = {}


# --------------------------------------------------------------------------
# Custom ACT table: hijack `sin` in silu_and_others to evaluate 6*N3(8u-4).
# Verified-on-HW stock mapping: ctrl entry = 42+(exp-116); entry 52 (binade
# [0.5,1)) has 8 sub-buckets of width 1/16 at buckets 1034..1041; bucket
# eval is y = d0+(u-x0)(d1+(u-x0)(d2+(u-x0)d3)); |u|<2^-11 -> bucket
# 1075/1076 (sign-folded); large |u| -> 1077/1078.  Buckets 1020..1078 are
# sin-private; everything else (silu, copy, ...) is untouched.
# --------------------------------------------------------------------------
def _n3_6_coeffs(j):
    return {
        0: [0.0, 0.0, 0.0, 1.0],
        1: [1.0, 3.0, 3.0, -3.0],
        2: [4.0, 0.0, -6.0, 3.0],
        3: [1.0, -3.0, 3.0, -1.0],
    }[j]


def _compose(c, scale, shift):
    c0, c1, c2, c3 = c
    return [
        c0 + c1 * shift + c2 * shift**2 + c3 * shift**3,
        scale * (c1 + 2 * c2 * shift + 3 * c3 * shift**2),
        scale**2 * (c2 + 3 * c3 * shift),
        scale**3 * c3,
    ]


def _build_custom_act_root():
    if "act_root" in _state:
        return _state["act_root"], _state["act_sig"]
    from neuronxcc.driver.Job import Job
    from neuronxcc.driver.jobs.support.FindActInfo import findActInfoFile

    src_json = findActInfoFile(Job.getPackageDir(), "gen3")
    src_dir = os.path.dirname(src_json)
    dst_dir = tempfile.mkdtemp(prefix="kan_act_root_")
    for f in os.listdir(src_dir):
        shutil.copy(os.path.join(src_dir, f), os.path.join(dst_dir, f))
    for f in os.listdir(dst_dir):
        os.chmod(os.path.join(dst_dir, f), 0o644)

    bkt_path = os.path.join(dst_dir, "silu_and_others_bkt.bin")
    bkt = np.fromfile(bkt_path, dtype=np.float32).reshape(-1, 8).copy()
    bkt[1020:1079] = 0.0
    for k in range(8):
        x0 = 0.5 + k / 16.0 + 1.0 / 32.0
        j = k // 2
        q = _compose(_n3_6_coeffs(j), 8.0, 8.0 * x0 - 4.0 - j)
        bkt[1034 + k] = [q[0], q[1], q[2], q[3], x0, 0.0, 0.0, 0.0]
    bkt.tofile(bkt_path)

    sig = hashlib.sha256(open(bkt_path, "rb").read()).hexdigest()[:10]
    path = os.path.join(dst_dir, "act_info.json")
    os.environ["BASS_ACT_ROOT_JSON_PATH"] = path
    _state["act_root"] = path
    _state["act_sig"] = sig
    return path, sig


# --------------------------------------------------------------------------
# Custom DVE ops
# --------------------------------------------------------------------------
def _register_ops():
    if "ops" in _state:
        return _state["ops"]
    import concourse.dve_ops as dve_ops
    from concourse.dve_spec import (
        Spec, Src0, Src1, C0, C1, C2, One, PageIdx, relu, sq, maxx, minn, lower,
    )
    from concourse.dve_uop import DveOpSpec

    def page_idx_np(in0, s0, s1):
        S = in0.shape[1]
        return (s0 + s1 * np.arange(S, dtype=np.float64)).astype(np.float32)[
            None, :, None
        ]

    def pre_ref(in0, in1, s0, s1, imm2):
        t = np.minimum(np.maximum(in0, np.float32(s0)), np.float32(s1))
        t = ((t + np.float32(1)) - np.float32(1)).astype(np.float32)
        return (t * np.float32(imm2)).astype(np.float32)

    def z_ref(in0, in1, s0, s1, imm2):
        t = (in0 + page_idx_np(in0, s0, s1)).astype(np.float32)
        m = np.minimum(t, np.float32(imm2) - t)
        zp = np.maximum(m, np.float32(0))
        d = (zp + zp).astype(np.float32)
        return ((d * d) * zp).astype(np.float32)

    def w_ref(in0, in1, s0, s1, imm2):
        t = (in0 + page_idx_np(in0, s0, s1)).astype(np.float32)
        m = np.minimum(t, np.float32(imm2) - t)
        wp = np.maximum(m, np.float32(0))
        ww = (wp * wp).astype(np.float32)
        return ((ww * wp) - in1).astype(np.float32)

    pre_spec = Spec(
        body=((minn(maxx(Src0, C0), C1) + One) - One) * C2, reference=pre_ref
    )
    # zp = relu(min(t-1, 3-t)) = relu(min(tz, 2-tz)), tz = xs + 4.5 - g;
    # z = 4*zp^3 via sq(zp+zp)*zp; w = relu(min(t,4-t))^3 - z.  No gamma
    # pre-scale needed -- both ops read xs directly.
    _pgz = PageIdx(C0, C1)
    _tz = Src0 + _pgz
    _zp = relu(minn(_tz, C2 - _tz))
    z_spec = Spec(body=sq(_zp + _zp) * _zp, reference=z_ref)
    _pgw = PageIdx(C0, C1)
    _tw = Src0 + _pgw
    _wp = relu(minn(_tw, C2 - _tw))
    w_spec = Spec(body=sq(_wp) * _wp - Src1, reference=w_ref)

    ops = {}
    for name, spec, subdim in (
        ("KAN_PRE", pre_spec, False),
        ("KAN_Z3", z_spec, True),
        ("KAN_W3", w_spec, True),
    ):
        if name in dve_ops._SUB_OPCODE_FOR_NAME:
            ops[name] = next(o for o in dve_ops.OPS if o.name == name)
            continue
        row = dve_ops._CUSTOM_DVE_ROW_BASE + len(dve_ops.OPS)
        assert row < 0x20, "custom-DVE row overflow"
        shas = {}
        for ver in ("v3", "v4"):
            try:
                tmp = DveOpSpec(
                    name=name, opcode=row, uops=lower(spec, ver=ver),
                    rd1_en=dve_ops.has_src1(spec),
                )
                shas[ver] = tmp.sha(ver)
            except Exception:
                pass
        op = dve_ops.DveOp(name, spec, subdim=subdim, uops_sha=shas)
        dve_ops.OPS.append(op)
        dve_ops._SUB_OPCODE_FOR_NAME[name] = row
        dve_ops.CUSTOM_DVE_SPECS[name] = spec
        ops[name] = op
    _state["ops"] = ops
    return ops


# --------------------------------------------------------------------------
# Kernel build
# --------------------------------------------------------------------------
def _build_kernel():
    if "nc" in _state:
        return _state["nc"]
    import concourse.bacc as bacc
    import concourse.mybir as mybir
    import concourse.tile as tile
    from concourse.bass import ts

    _build_custom_act_root()
    ops = _register_ops()
    f32 = mybir.dt.float32
    bf16 = mybir.dt.bfloat16
    AF = mybir.ActivationFunctionType

    nc = bacc.Bacc()

    # x is shipped bf16 (halves input DMA traffic; adds ~5e-4 rel err) and
    # V partition-major so every DMA slice is contiguous per partition.
    fp8 = mybir.dt.float8e4
    xT = nc.dram_tensor("xT", [IN_F, BPC], bf16, kind="ExternalInput")
    V = nc.dram_tensor("V", [128, KC * OUT_F], bf16, kind="ExternalInput")
    # fp8 copies of the (g=6,7) V rows for input tiles it=2,3: one
    # DoubleRow matmul replaces two bf16 chunk-matmuls there (~1.2e-2
    # added rel err per the e4m3 simulation; gate is 2e-2)
    V8 = nc.dram_tensor("V8", [128, 2 * 2 * OUT_F], fp8, kind="ExternalInput")
    yT = nc.dram_tensor("yT", [OUT_F, BPC], bf16, kind="ExternalOutput")

    # V DMA slices (in units of K-chunks, consumption order), all on
    # gpsimd's software-dynamic queue: fine-grained early slices so chunk r
    # lands before the first-step matmul that consumes it.  Slice 0 is the
    # queue's first op (it absorbs the queue spin-up itself).
    # Slice pacing (all on gpsimd -- sharing the sync queue pushes the x
    # tiles late and cascades): measured gpsimd completions land at
    # ~10.1us + 0.67us/extra-chunk for the first slice, ~12.4us for the
    # second, then ~0.7us apart.  A 2-chunk first slice covers chunks 0-1
    # by ~10.8us (consumption deadlines 10.8, 11.9) without delaying the
    # first matmul the way a 3-chunk slice does.
    V_SLICES_GP = [
        (0, 2), (2, 4), (4, 6), (6, 8), (8, 16), (16, 24), (24, 32),
    ]

    def flush_one(nc, ysb_pool, acc, o, bs, split):
        ysb = ysb_pool.tile([128, BS], bf16, name=f"ysb{o}")
        if split:
            for hc0, hc1 in ((0, BS // 2), (BS // 2, BS)):
                nc.scalar.copy(ysb[:, hc0:hc1], acc[:, hc0:hc1])
                nc.sync.dma_start(
                    yT[ts(o, 128), bs * BS + hc0 : bs * BS + hc1],
                    ysb[:, hc0:hc1],
                )
        else:
            nc.scalar.copy(ysb[:], acc[:])
            nc.sync.dma_start(yT[ts(o, 128), ts(bs, BS)], ysb[:])

    with tile.TileContext(nc) as tc:
        with (
            tc.tile_pool(name="vpool", bufs=1) as vpool,
            tc.tile_pool(name="const", bufs=1) as const_pool,
            tc.tile_pool(name="warm", bufs=1) as warm_pool,
            tc.tile_pool(name="xin", bufs=3) as xin_pool,
            tc.tile_pool(name="xs", bufs=3) as xs_pool,
            tc.tile_pool(name="z3", bufs=2) as z3_pool,
            tc.tile_pool(name="feat", bufs=8) as feat_pool,
            tc.tile_pool(name="ft8", bufs=4) as ft8_pool,
            tc.tile_pool(name="ysb", bufs=4) as ysb_pool,
            tc.tile_pool(name="psum", bufs=8, space="PSUM") as psum_pool,
        ):
            v_sb = vpool.tile([128, KC, OUT_F], bf16)
            v_view = V[:].rearrange("p (kc o) -> p kc o", kc=KC)
            v8_sb = vpool.tile([128, 2, 2, OUT_F], fp8, name="v8")
            v8_view = V8[:].rearrange("p (i k o) -> p i k o", i=2, k=2)

            # The first x half-tile is the sync hardware queue's FIRST op:
            # its completion semaphore gates the whole feature->matmul chain.
            FIRST_HALVES = [(0, BS // 2), (BS // 2, BS)]
            xin0 = xin_pool.tile([128, BS], bf16, name="xin0")
            for c0, c1 in FIRST_HALVES:
                nc.sync.dma_start(xin0[:, c0:c1], xT[0:128, c0:c1])

            # warm-matmul scratch memset on gpsimd (its earliest user slot),
            # then the V stream.
            warm = warm_pool.tile([128, 129], bf16, name="warmw")
            nc.gpsimd.memset(warm[:, 0:128], 0.0)
            for a, b in V_SLICES_GP[:3]:
                nc.gpsimd.dma_start(v_sb[:, a:b, :], v_view[:, a:b, :])
            nc.gpsimd.dma_start(v8_sb[:], v8_view[:])
            for a, b in V_SLICES_GP[3:]:
                nc.gpsimd.dma_start(v_sb[:, a:b, :], v_view[:, a:b, :])

            # Kick the ACT table load for silu_and_others immediately so it
            # overlaps the input DMAs instead of the first feature chain.
            cb = const_pool.tile([128, NG + 1], f32, name="cbias")
            nc.vector.memset(cb[:, NG : NG + 1], 0.0)
            for g in range(N_DVE, NG):
                nc.vector.memset(cb[:, g : g + 1], (9.5 - g) / 8.0)
            nc.vector.memset(warm[:, 128:129], 0.0)
            nc.scalar.activation(
                warm[:, 128:129], warm[:, 128:129], AF.Silu, bias=cb[:, NG : NG + 1]
            )

            # PE HAM warm-up: throwaway matmuls into a scratch PSUM tile so
            # the clock-gate lifts before the real matmul stream begins.
            warm_ps = psum_pool.tile([128, BS], f32, name="warmps", tag="acc")
            for _ in range(N_WARM):
                nc.tensor.matmul(
                    warm_ps[:, 0:128], warm[:, 0:128], warm[:, 0:128],
                    start=True, stop=True,
                )

            pending = None  # (accs, bs) whose y copies are deferred
            for bs in range(N_BS):
                accs = [
                    psum_pool.tile([128, BS], f32, name=f"acc{o}", tag="acc")
                    for o in range(N_IT)
                ]
                for it in range(N_IT):
                    first_step = bs == 0 and it == 0
                    last_step = bs == N_BS - 1 and it == N_IT - 1
                    halves = FIRST_HALVES if first_step else [(0, BS)]
                    if first_step:
                        xin = xin0  # DMA'd in the preamble slot above
                    else:
                        xin = xin_pool.tile([128, BS], bf16)
                        nc.sync.dma_start(xin[:], xT[ts(it, 128), ts(bs, BS)])
                    xs = xs_pool.tile([128, BS], f32)
                    for c0, c1 in halves:
                        nc.vector._custom_dve(
                            ops["KAN_PRE"], out=xs[:, c0:c1], in0=xin[:, c0:c1],
                            s0=-1.1, s1=1.1, imm2=2.5,
                        )
                    # steady steps on the upper input tiles run bases 6,7 as
                    # one fp8 DoubleRow matmul; their SINs emit fp8 directly
                    dr_step = it >= 2 and not last_step
                    ft = feat_pool.tile([128, NG, BS], bf16)
                    ft8 = (
                        ft8_pool.tile([128, 2, BS], fp8, name="ft8")
                        if dr_step else None
                    )
                    # bases N_DVE..7: one ACT spline-table op each (half-0's
                    # five SINs all before half-1's, matching MM consumption)
                    for c0, c1 in halves:
                        for g in range(N_DVE, NG):
                            if dr_step and g >= 6:
                                nc.scalar.activation(
                                    ft8[:, g - 6, c0:c1], xs[:, c0:c1],
                                    AF.Sin, scale=0.125,
                                    bias=cb[:, g : g + 1],
                                )
                            else:
                                nc.scalar.activation(
                                    ft[:, g, c0:c1], xs[:, c0:c1], AF.Sin,
                                    scale=0.125, bias=cb[:, g : g + 1],
                                )
                    # bases 0..N_DVE-1: two fused paged DVE ops (z = 4*zp^3,
                    # w = wp^3 - z), both directly on xs
                    z3 = z3_pool.tile([128, N_DVE, BS], f32)
                    nc.vector._custom_dve(
                        ops["KAN_Z3"],
                        out=z3[:],
                        in0=xs[:].unsqueeze(1).broadcast_to([128, N_DVE, BS]),
                        s0=4.5, s1=-1.0, imm2=2.0,
                    )
                    nc.vector._custom_dve(
                        ops["KAN_W3"],
                        out=ft[:, 0:N_DVE, :],
                        in0=xs[:].unsqueeze(1).broadcast_to([128, N_DVE, BS]),
                        in1=z3[:].rearrange("p s n -> p (s n)"),
                        s0=5.5, s1=-1.0, imm2=4.0,
                    )
                    if last_step:
                        # o-major, column halves; flush each acc[o] as soon
                        # as its own accumulation stops so the PSUM drain
                        # pipelines with the remaining o-tiles' matmuls.
                        # The last o additionally flushes half-0 before
                        # half-1's matmuls run.
                        for o in range(N_IT):
                            last_o = o == N_IT - 1
                            for hc0, hc1 in ((0, BS // 2), (BS // 2, BS)):
                                for r in range(NG):
                                    g = CHUNK_ORDER[r]
                                    nc.tensor.matmul(
                                        accs[o][:, hc0:hc1],
                                        v_sb[:, it * NG + r, ts(o, 128)],
                                        ft[:, g, hc0:hc1],
                                        start=False,
                                        stop=(hc0 > 0 and r == NG - 1),
                                        skip_group_check=True,
                                    )
                            if last_o:
                                # drain via the idle DVE (reacts faster
                                # than ScalarE's queued COPY after the
                                # final matmul); single copy+DMA, no
                                # column split -- a partial read of the
                                # still-accumulating tile serializes the
                                # remaining matmuls (coarse PSUM tracking)
                                ysb3 = ysb_pool.tile(
                                    [128, BS], bf16, name="ysb3"
                                )
                                nc.vector.tensor_copy(ysb3[:], accs[o][:])
                                nc.sync.dma_start(
                                    yT[ts(o, 128), ts(bs, BS)], ysb3[:]
                                )
                            else:
                                flush_one(nc, ysb_pool, accs[o], o, bs,
                                          split=False)
                        continue
                    if first_step:
                        # column-half-major: all ACT ranks of half-0 for all
                        # o first (half-1 features and the DVE chunks are
                        # still being produced), then half-1, then the DVE
                        # ranks full-width.  start=True on each acc's first
                        # MM clears the whole bank, so the later half-1 /
                        # full-width MMs accumulate correctly.
                        for hi, (c0, c1) in enumerate(halves):
                            for r, g in enumerate(CHUNK_ORDER):
                                if g < N_DVE:
                                    continue
                                for o in range(N_IT):
                                    nc.tensor.matmul(
                                        accs[o][:, c0:c1],
                                        v_sb[:, it * NG + r, ts(o, 128)],
                                        ft[:, g, c0:c1],
                                        start=(hi == 0 and r == 0),
                                        stop=False,
                                        skip_group_check=True,
                                    )
                        for r, g in enumerate(CHUNK_ORDER):
                            if g >= N_DVE:
                                continue
                            for o in range(N_IT):
                                nc.tensor.matmul(
                                    accs[o][:],
                                    v_sb[:, it * NG + r, ts(o, 128)],
                                    ft[:, g, :],
                                    start=False, stop=False,
                                    skip_group_check=True,
                                )
                    else:
                        # r-major: all 4 o-tiles of a K-chunk back-to-back,
                        # so the DVE-produced chunks (ranks 5..7) aren't
                        # needed until ~4.3us into the step.
                        for r, g in enumerate(CHUNK_ORDER):
                            if dr_step and r == 4:
                                continue  # fused into the r==3 DoubleRow MM
                            for o in range(N_IT):
                                if dr_step and r == 3:
                                    nc.tensor.matmul(
                                        accs[o][:],
                                        v8_sb[:, it - 2, :, ts(o, 128)],
                                        ft8[:],
                                        start=False, stop=False,
                                        perf_mode=mybir.MatmulPerfMode.DoubleRow,
                                        skip_group_check=True,
                                    )
                                    continue
                                nc.tensor.matmul(
                                    accs[o][:],
                                    v_sb[:, it * NG + r, ts(o, 128)],
                                    ft[:, g, :],
                                    start=(it == 0 and r == 0),
                                    stop=(it == N_IT - 1 and r == NG - 1),
                                    skip_group_check=True,
                                )
                    if it == 0 and pending is not None:
                        paccs, pbs = pending
                        for o in range(N_IT):
                            flush_one(nc, ysb_pool, paccs[o], o, pbs, split=False)
                        pending = None
                pending = (accs, bs)
            # bs == N_BS-1 was flushed inside last_step

    nc.compile()
    _state["nc"] = nc
    return nc


def _silu_in_basis():
    """Project silu(x) on [-1.1, 1.1] onto the 8 B-spline bases, weighted by
    the clipped-N(0,1) input distribution (atoms at the clamp bounds)."""
    from math import erf, sqrt

    def n3(t):
        wp = np.maximum(np.minimum(t, 4 - t), 0.0)
        zp = np.maximum(np.minimum(t - 1, 3 - t), 0.0)
        return (wp**3 - 4 * zp**3) / 6.0

    x = np.linspace(-1.0999, 1.0999, 8001)
    w = np.exp(-x**2 / 2) / np.sqrt(2 * np.pi) * (x[1] - x[0])
    tail = 1 - 0.5 * (1 + erf(1.1 / sqrt(2)))
    X = np.concatenate([x, [-1.1, 1.1]])
    W = np.concatenate([w, [tail, tail]])
    s = 2.5 * X + 5.5
    Bm = np.stack([n3(s - g) for g in range(NG)], axis=-1)
    F = X / (1 + np.exp(-X))
    swr = np.sqrt(W)
    c, *_ = np.linalg.lstsq(Bm * swr[:, None], F * swr, rcond=None)
    return c  # (8,)


def _build_V(base_weight, spline_weight, spline_scaler):
    sw = spline_weight.astype(np.float32) * spline_scaler.astype(np.float32)[:, :, None]
    vs = np.transpose(sw, (2, 1, 0)) / np.float32(6.0)  # [g, i, o]
    bwT = base_weight.astype(np.float32).T  # [i, o]
    c = _silu_in_basis() / 6.0
    V = np.empty((KC * 128, OUT_F), dtype=np.float32)
    for it in range(N_IT):
        isl = slice(it * 128, (it + 1) * 128)
        for r, g in enumerate(CHUNK_ORDER):
            k = it * NG + r
            V[k * 128 : (k + 1) * 128] = vs[g, isl, :] + np.float32(c[g]) * bwT[isl, :]
    # partition-major: [p, kc, o] so each DMA slice is per-partition contiguous
    Vp = V.reshape(KC, 128, OUT_F).transpose(1, 0, 2).reshape(128, KC * OUT_F)
    import ml_dtypes
    return np.ascontiguousarray(Vp.astype(ml_dtypes.bfloat16))


def _build_V8(base_weight, spline_weight, spline_scaler):
    sw = spline_weight.astype(np.float32) * spline_scaler.astype(np.float32)[:, :, None]
    vs = np.transpose(sw, (2, 1, 0)) / np.float32(6.0)  # [g, i, o]
    bwT = base_weight.astype(np.float32).T
    c = _silu_in_basis() / 6.0
    out = np.empty((128, 2, 2, OUT_F), dtype=np.float32)
    for itp, it in enumerate((2, 3)):
        isl = slice(it * 128, (it + 1) * 128)
        for ko, g in enumerate((6, 7)):
            out[:, itp, ko, :] = vs[g, isl, :] + np.float32(c[g]) * bwT[isl, :]
    import ml_dtypes
    return np.ascontiguousarray(
        out.reshape(128, -1).astype(ml_dtypes.float8_e4m3)
    )


def kernel(x, base_weight, spline_weight, spline_scaler, grid):
    from concourse.bass_utils import run_bass_kernel_spmd

    import ml_dtypes

    nc = _build_kernel()
    Vb = _build_V(base_weight, spline_weight, spline_scaler)
    V8b = _build_V8(base_weight, spline_weight, spline_scaler)
    x = np.asarray(x, dtype=np.float32)
    in_maps = []
    for c in range(N_CORES):
        xTc = np.ascontiguousarray(
            x[c * BPC : (c + 1) * BPC, :].T.astype(ml_dtypes.bfloat16)
        )
        in_maps.append({"xT": xTc, "V": Vb, "V8": V8b})
    res = run_bass_kernel_spmd(nc, in_maps, core_ids=list(range(N_CORES)))
    y = np.empty((B, OUT_F), dtype=np.float32)
    for c in range(N_CORES):
        y[c * BPC : (c + 1) * BPC, :] = res.results[c]["yT"].T
    return y
